# revision 1
# baseline (speedup 1.0000x reference)
"""Trainium2 Bass kernel for nn_CHGANSimplified (sparse graph attention).

Math (per batch b, time t):
  enh = x + type_embed[parity(n)]
  Q/K/V = enh @ W*.T + b*          (4 heads, head dim 32)
  S_h = (Q_h K_h^T)/sqrt(32) + edge_bias ; masked where adj==0 & ~eye
  out = LN(concat_h(softmax(S_h) V_h) @ Wo.T + bo + x)

Device strategy (8 cores, data-parallel over the 24 (b,t) pairs, 3 each):
  - everything in a feature-on-partition transposed layout:
      enhT (D=128 part, N=1024), QT/KT = W^T-stationary matmuls over enhT
  - scores: st[m_tile, nq] = K_h(m)·Q_h(nq)  via K=32 matmuls (tile_position
    rows 32h), i.e. S^T blocks of (128 m, 1024 nq) in PSUM
  - softmax without max-subtraction (scores are O(5)): exp on ACT straight
    from PSUM; multiplicative mask (host precomputes exp(edge_bias)*keep,
    exactly 0/1 here) applied on DVE
  - AV: expST tile is the PE stationary; moving = [V_h | ones] (128x33)
    -> natural-layout (nq, 32) output + softmax denominator in col 32
  - normalize by reciprocal(denom) per partition, PE-transpose the
    concatenated heads, Wo projection, residual + LayerNorm, DMA out.
"""

import os
import sys

sys.path.insert(0, "/opt/trn_rl_repo")

from contextlib import ExitStack

import ml_dtypes
import numpy as np

import concourse.bass as bass
import concourse.tile as tile
from concourse import bacc, mybir
from concourse.bass_utils import run_bass_kernel_spmd

B, N, T, D, H, DH = 2, 1024, 12, 128, 4, 32
NCORES = 8
PAIRS = [(b, t) for b in range(B) for t in range(T)]
PER_CORE = len(PAIRS) // NCORES  # 3
EPS = 1e-5
NTILE = N // 128  # 8

_DT_CFG = os.environ.get("BASSK_DT", "bf16")
if _DT_CFG == "bf16":
    MM_DT, MM_NP = mybir.dt.bfloat16, ml_dtypes.bfloat16
elif _DT_CFG == "f32r":
    MM_DT, MM_NP = mybir.dt.float32r, np.float32
else:
    MM_DT, MM_NP = mybir.dt.float32, np.float32

F32 = mybir.dt.float32
AF = mybir.ActivationFunctionType

# mask routing: m-tiles < INJ get the additive mask injected on PE;
# remaining alternate DVE / Pool (Pool only if BASSK_POOL=1)
INJ = int(os.environ.get("BASSK_INJ", "4"))
USE_POOL = bool(int(os.environ.get("BASSK_POOL", "1")))

LAST_RESULTS = None  # BassKernelResults of the most recent run (for test.py)


def _build_nc():
    nc = bacc.Bacc()

    xT_d = nc.dram_tensor("xt", [PER_CORE, 128, N], F32, kind="ExternalInput")
    xpb_d = nc.dram_tensor("xpb", [PER_CORE, N, D], F32, kind="ExternalInput")
    ta_d = nc.dram_tensor("ta", [128, N], F32, kind="ExternalInput")
    wq_d = nc.dram_tensor("wq", [D, D], MM_DT, kind="ExternalInput")
    wk_d = nc.dram_tensor("wk", [D, D], MM_DT, kind="ExternalInput")
    wv_d = nc.dram_tensor("wv", [D, D], MM_DT, kind="ExternalInput")
    wo_d = nc.dram_tensor("wo", [D, D], MM_DT, kind="ExternalInput")
    bq_d = nc.dram_tensor("bq", [D, 1], F32, kind="ExternalInput")
    bk_d = nc.dram_tensor("bk", [D, 1], F32, kind="ExternalInput")
    bvb_d = nc.dram_tensor("bvb", [128, D], F32, kind="ExternalInput")
    lng_d = nc.dram_tensor("lng", [128, D], F32, kind="ExternalInput")
    lnb_d = nc.dram_tensor("lnb", [128, D], F32, kind="ExternalInput")
    id_d = nc.dram_tensor("ident", [128, 128], MM_DT, kind="ExternalInput")
    # mask, transposed (m, nq): additive (PE psum-injection) + multiplicative
    # (DVE/Pool post-exp) variants; each m-tile loads from one of them
    maska_d = nc.dram_tensor("maska", [N, N], MM_DT, kind="ExternalInput")
    maskm_d = nc.dram_tensor("maskm", [N, N], MM_DT, kind="ExternalInput")
    out_d = nc.dram_tensor("out", [PER_CORE, N, D], F32, kind="ExternalOutput")

    with tile.TileContext(nc) as tc, ExitStack() as ctx:
        const = ctx.enter_context(tc.tile_pool(name="const", bufs=1))
        work = ctx.enter_context(tc.tile_pool(name="work", bufs=2))
        expp = ctx.enter_context(tc.tile_pool(name="expp", bufs=16))
        pst = ctx.enter_context(tc.tile_pool(name="pst", bufs=2, space="PSUM"))
        pav = ctx.enter_context(tc.tile_pool(name="pav", bufs=2, space="PSUM"))
        pac = ctx.enter_context(tc.tile_pool(name="pac", bufs=2, space="PSUM"))

        # ---- constants ----
        ta_sb = const.tile([128, N], F32)
        nc.gpsimd.dma_start(ta_sb, ta_d[:, :])
        wq_sb = const.tile([D, D], MM_DT)
        nc.gpsimd.dma_start(wq_sb, wq_d[:, :])
        wk_sb = const.tile([D, D], MM_DT)
        nc.gpsimd.dma_start(wk_sb, wk_d[:, :])
        wv_sb = const.tile([D, D], MM_DT)
        nc.gpsimd.dma_start(wv_sb, wv_d[:, :])
        wo_sb = const.tile([D, D], MM_DT)
        nc.gpsimd.dma_start(wo_sb, wo_d[:, :])
        bq_sb = const.tile([D, 1], F32)
        nc.gpsimd.dma_start(bq_sb, bq_d[:, :])
        bk_sb = const.tile([D, 1], F32)
        nc.gpsimd.dma_start(bk_sb, bk_d[:, :])
        bvb_sb = const.tile([128, D], F32)
        nc.gpsimd.dma_start(bvb_sb, bvb_d[:, :])
        lng_sb = const.tile([128, D], F32)
        nc.gpsimd.dma_start(lng_sb, lng_d[:, :])
        lnb_sb = const.tile([128, D], F32)
        nc.gpsimd.dma_start(lnb_sb, lnb_d[:, :])
        id_sb = const.tile([128, 128], MM_DT)
        nc.gpsimd.dma_start(id_sb, id_d[:, :])
        eps_sb = const.tile([128, 1], F32)
        nc.vector.memset(eps_sb, EPS)
        mask_sb = []
        for m in range(NTILE):
            mt = const.tile([128, N], MM_DT, name=f"mask{m}", tag=f"mask{m}")
            src = maska_d if m < INJ else maskm_d
            nc.gpsimd.dma_start(mt, src[m * 128 : (m + 1) * 128, :])
            mask_sb.append(mt)

        for it in range(PER_CORE):
            # ---- load + enhance ----
            xT_sb = work.tile([128, N], F32, name=f"xT{it}", tag="xT")
            nc.gpsimd.dma_start(xT_sb, xT_d[it])
            xpb_sb = work.tile([128, NTILE, D], F32, name=f"xpb{it}", tag="xpb")
            nc.gpsimd.dma_start(xpb_sb, xpb_d[it].rearrange("(q p) d -> p q d", p=128))
            enhT = work.tile([128, N], MM_DT, name=f"enhT{it}", tag="enhT")
            nc.vector.tensor_add(enhT, xT_sb, ta_sb)

            # ---- projections: QT/KT (feature-major) ----
            # split into lo/hi 64-partition tiles: head slices must start at
            # partition offset 0/32 (HW limit: base partition in {0,32,64})
            qt_lo = work.tile([64, N], MM_DT, name=f"qtl{it}", tag="qtl")
            qt_hi = work.tile([64, N], MM_DT, name=f"qth{it}", tag="qth")
            kt_lo = work.tile([64, N], MM_DT, name=f"ktl{it}", tag="ktl")
            kt_hi = work.tile([64, N], MM_DT, name=f"kth{it}", tag="kth")
            for dlo, dhi, w_sb, b_sb in (
                (qt_lo, qt_hi, wq_sb, bq_sb),
                (kt_lo, kt_hi, wk_sb, bk_sb),
            ):
                ps = pst.tile([128, N], F32, name=f"ps{it}", tag="st")
                for j in range(2):
                    nc.tensor.matmul(
                        ps[:, j * 512 : (j + 1) * 512],
                        w_sb,
                        enhT[:, j * 512 : (j + 1) * 512],
                        start=True,
                        stop=True,
                    )
                nc.vector.tensor_scalar_add(dlo, ps[0:64, :], b_sb[0:64, :])
                nc.vector.tensor_scalar_add(dhi, ps[64:128, :], b_sb[64:128, :])

            # ---- V natural + augmented with ones column ----
            vaugs = []
            for m in range(NTILE):
                vps = pav.tile([128, D], F32, name=f"vps{it}_{m}", tag="pv")
                nc.tensor.matmul(
                    vps, enhT[:, m * 128 : (m + 1) * 128], wv_sb, start=True, stop=True
                )
                va = work.tile(
                    [128, H, DH + 1], MM_DT, name=f"vaug{it}_{m}", tag=f"vaug{m}"
                )
                nc.gpsimd.memset(va, 1.0)
                nc.vector.tensor_add(
                    va[:, :, 0:DH],
                    vps.rearrange("p (h d) -> p h d", h=H),
                    bvb_sb.rearrange("p (h d) -> p h d", h=H),
                )
                vaugs.append(va)

            onats = [
                work.tile([128, D], MM_DT, name=f"onat{it}_{q}", tag=f"onat{q}")
                for q in range(NTILE)
            ]

            # ---- attention per head ----
            for h in range(H):
                hs = slice(h * DH, (h + 1) * DH)
                qt_t = qt_lo if h < 2 else qt_hi
                kt_t = kt_lo if h < 2 else kt_hi
                po = 32 * (h % 2)
                es = []
                for m in range(NTILE):
                    st = pst.tile([128, N], F32, name=f"st{it}_{h}_{m}", tag="st")
                    inject = m < INJ  # additive mask via PE psum accumulation
                    if inject:
                        for j in range(2):
                            nc.tensor.matmul(
                                st[:, j * 512 : (j + 1) * 512],
                                id_sb,
                                mask_sb[m][:, j * 512 : (j + 1) * 512],
                                start=True,
                                stop=False,
                            )
                    for j in range(2):
                        nc.tensor.matmul(
                            st[:, j * 512 : (j + 1) * 512],
                            kt_t[po : po + 32, m * 128 : (m + 1) * 128],
                            qt_t[po : po + 32, j * 512 : (j + 1) * 512],
                            start=not inject,
                            stop=True,
                        )
                    e = expp.tile([128, N], MM_DT, name=f"e{it}_{h}_{m}", tag="expst")
                    nc.scalar.activation(e, st, AF.Exp)
                    if not inject:
                        if USE_POOL and (m - INJ) % 2 == 1:
                            nc.gpsimd.tensor_mul(e, e, mask_sb[m])
                        else:
                            nc.vector.tensor_mul(e, e, mask_sb[m])
                    es.append(e)
                for q in range(NTILE):
                    av = pac.tile([128, DH + 1], F32, name=f"av{it}_{h}_{q}", tag="av")
                    for m in range(NTILE):
                        nc.tensor.matmul(
                            av,
                            es[m][:, q * 128 : (q + 1) * 128],
                            vaugs[m][:, h, :],
                            start=(m == 0),
                            stop=(m == NTILE - 1),
                        )
                    rec = work.tile([128, 1], F32, name=f"rec{it}_{h}_{q}", tag="rec", bufs=8)
                    nc.vector.reciprocal(rec, av[:, DH : DH + 1])
                    nc.vector.tensor_scalar_mul(onats[q][:, hs], av[:, 0:DH], rec)

            # ---- transpose heads-concat output, project, residual + LN ----
            ot = work.tile([128, N], MM_DT, name=f"ot{it}", tag="ot")
            for q in range(NTILE):
                tp = pav.tile([128, 128], MM_DT, name=f"tp{it}_{q}", tag="pv")
                nc.tensor.transpose(tp, onats[q], id_sb)
                nc.vector.tensor_copy(ot[:, q * 128 : (q + 1) * 128], tp)

            y = work.tile([128, NTILE, D], F32, name=f"y{it}", tag="y")
            mv = work.tile([128, NTILE, 2], F32, name=f"mv{it}", tag="mv")
            for q in range(NTILE):
                op = pav.tile([128, D], F32, name=f"op{it}_{q}", tag="pv")
                nc.tensor.matmul(
                    op, ot[:, q * 128 : (q + 1) * 128], wo_sb, start=True, stop=True
                )
                nc.vector.tensor_add(y[:, q, :], op, xpb_sb[:, q, :])
                st6 = work.tile([128, 6], F32, name=f"st6{it}_{q}", tag="st6", bufs=8)
                nc.vector.bn_stats(st6, y[:, q, :])
                nc.vector.bn_aggr(mv[:, q, :], st6)
            sd = work.tile([128, NTILE, 1], F32, name=f"sd{it}", tag="sd")
            nc.scalar.activation(sd, mv[:, :, 1:2], AF.Sqrt, bias=eps_sb[:, 0:1])
            rstd = work.tile([128, NTILE, 1], F32, name=f"rstd{it}", tag="rstd")
            nc.vector.reciprocal(rstd, sd)
            oall = work.tile([128, NTILE, D], F32, name=f"oall{it}", tag="oall")
            for q in range(NTILE):
                z = work.tile([128, D], F32, name=f"z{it}_{q}", tag="z", bufs=4)
                nc.vector.tensor_scalar(
                    z,
                    y[:, q, :],
                    mv[:, q, 0:1],
                    rstd[:, q, 0:1],
                    op0=mybir.AluOpType.subtract,
                    op1=mybir.AluOpType.mult,
                )
                nc.vector.tensor_mul(z, z, lng_sb)
                nc.vector.tensor_add(oall[:, q, :], z, lnb_sb)
            nc.gpsimd.dma_start(out_d[it].rearrange("(q p) d -> p q d", p=128), oall)

    nc.compile()
    return nc


_nc_cache = {}


def _get_nc():
    key = _DT_CFG
    if key not in _nc_cache:
        _nc_cache[key] = _build_nc()
    return _nc_cache[key]


def kernel(
    node_features,
    adj_mx,
    node_type_embed,
    Wq,
    bq,
    Wk,
    bk,
    Wv,
    bv,
    edge_bias,
    Wo,
    bo,
    ln_g,
    ln_b,
):
    global LAST_RESULTS
    nf = np.asarray(node_features, np.float32)
    adj = np.asarray(adj_mx)
    nte = np.asarray(node_type_embed, np.float32)
    Wq = np.asarray(Wq, np.float32)
    Wk = np.asarray(Wk, np.float32)
    Wv = np.asarray(Wv, np.float32)
    Wo = np.asarray(Wo, np.float32)
    bq = np.asarray(bq, np.float32)
    bk = np.asarray(bk, np.float32)
    bv = np.asarray(bv, np.float32)
    bo = np.asarray(bo, np.float32)
    edge_bias = np.asarray(edge_bias, np.float32)
    ln_g = np.asarray(ln_g, np.float32)
    ln_b = np.asarray(ln_b, np.float32)

    scale = 1.0 / np.sqrt(DH)

    # shared (replicated) inputs
    types = 1 - (np.arange(N) % 2)
    ta = np.ascontiguousarray(nte[types].T)  # (D, N)
    keep = np.maximum(adj.astype(np.float32), np.eye(N, dtype=np.float32))
    maskmul = np.ascontiguousarray((np.exp(edge_bias) * keep).T)  # (m, nq)
    maskadd = np.ascontiguousarray((edge_bias + (keep - 1.0) * 1e30).T)
    shared = {
        "ta": ta,
        "wq": np.ascontiguousarray(Wq.T * scale).astype(MM_NP),
        "wk": np.ascontiguousarray(Wk.T).astype(MM_NP),
        "wv": np.ascontiguousarray(Wv.T).astype(MM_NP),
        "wo": np.ascontiguousarray(Wo.T).astype(MM_NP),
        "bq": np.ascontiguousarray((bq * scale).reshape(D, 1)),
        "bk": np.ascontiguousarray(bk.reshape(D, 1)),
        "bvb": np.ascontiguousarray(np.broadcast_to(bv, (128, D))),
        "lng": np.ascontiguousarray(np.broadcast_to(ln_g, (128, D))),
        "lnb": np.ascontiguousarray(np.broadcast_to(ln_b, (128, D))),
        "ident": np.eye(128, dtype=MM_NP),
        "maska": maskadd.astype(MM_NP),
        "maskm": maskmul.astype(MM_NP),
    }

    in_maps = []
    for c in range(NCORES):
        pairs = PAIRS[c * PER_CORE : (c + 1) * PER_CORE]
        xT = np.stack([np.ascontiguousarray(nf[b, :, t, :].T) for (b, t) in pairs])
        xpb = np.stack([nf[b, :, t, :] + bo for (b, t) in pairs])
        in_maps.append({**shared, "xt": xT, "xpb": xpb})

    nc = _get_nc()
    res = run_bass_kernel_spmd(
        nc,
        in_maps,
        core_ids=list(range(NCORES)),
        trace=bool(int(os.environ.get("BASSK_TRACE", "0"))),
    )
    LAST_RESULTS = res

    out = np.empty((B, N, T, D), np.float32)
    for c in range(NCORES):
        pairs = PAIRS[c * PER_CORE : (c + 1) * PER_CORE]
        for i, (b, t) in enumerate(pairs):
            out[b, :, t, :] = res.results[c]["out"][i]
    return out



# revision 2
# speedup vs baseline: 1.1352x; 1.1352x over previous
"""Trainium2 Bass kernel for nn_CHGANSimplified (sparse graph attention).

Math (per batch b, time t):
  enh = x + type_embed[parity(n)]
  Q/K/V = enh @ W*.T + b*          (4 heads, head dim 32)
  S_h = (Q_h K_h^T)/sqrt(32) + edge_bias ; masked where adj==0 & ~eye
  out = LN(concat_h(softmax(S_h) V_h) @ Wo.T + bo + x)

Device strategy (8 cores, data-parallel over the 24 (b,t) pairs, 3 each).
Per-core schedule (the ACT engine's exp stream is the critical resource;
everything else is shaped to keep it saturated):

  - feature-major layout throughout: enhT/qt/kt (D=128 part, N=1024);
    head h occupies partitions 32h..32h+31 "for free".
  - QK^T: 4x row-tiled PE (K=32 per head, tile_position=(32h,0)), st
    psum [128, 2 heads x 512]; exp straight off PSUM on ACT -> e bf16;
    multiplicative mask (keep 0/1 incl. diag) on DVE.
  - AV: col-tiled PE (V_h natural [m,32] stationary, tile_position=
    (0,32h)) accumulating over m -> output lands feature-major, no
    transposes. Denominator via all-ones [128,32] stationary in the
    same col-tiled pass -> per-lane broadcast rowsums; one reciprocal +
    one mul normalizes.
  - Wo: stationary-swap (evT chunk stationary, Wo^T moving) -> output
    directly natural (nq part, D free) for residual + LayerNorm.
  - LN sqrt deferred to a single tail after all pairs (one ACT table
    switch total).
  - software pipeline at half-pair (j = nq half) granularity: AV/out of
    pair p runs in PE slack under the exp stream of pair p/p+1.
"""

import os
import sys

sys.path.insert(0, "/opt/trn_rl_repo")

from contextlib import ExitStack

import ml_dtypes
import numpy as np

import concourse.bass as bass
import concourse.tile as tile
from concourse import bacc, mybir
from concourse.bass_utils import run_bass_kernel_spmd

B, N, T, D, H, DH = 2, 1024, 12, 128, 4, 32
NCORES = 8
PAIRS = [(b, t) for b in range(B) for t in range(T)]
PER_CORE = len(PAIRS) // NCORES  # 3
EPS = 1e-5
NTILE = N // 128  # 8

BF16, BF16_NP = mybir.dt.bfloat16, ml_dtypes.bfloat16
F32 = mybir.dt.float32
AF = mybir.ActivationFunctionType

# number of (hh,m) mask-muls per (pair,j) routed to gpsimd (of 16)
POOLM = int(os.environ.get("BASSK_POOLM", "0"))

LAST_RESULTS = None  # BassKernelResults of the most recent run (for test.py)


def _build_nc():
    nc = bacc.Bacc()

    xT_d = nc.dram_tensor("xt", [PER_CORE, 128, N], F32, kind="ExternalInput")
    xpb_d = nc.dram_tensor("xpb", [PER_CORE, N, D], F32, kind="ExternalInput")
    ta_d = nc.dram_tensor("ta", [128, N], F32, kind="ExternalInput")
    wq_d = nc.dram_tensor("wq", [D, D], BF16, kind="ExternalInput")
    wk_d = nc.dram_tensor("wk", [D, D], BF16, kind="ExternalInput")
    wv_d = nc.dram_tensor("wv", [D, D], BF16, kind="ExternalInput")
    wo_d = nc.dram_tensor("wo", [D, D], BF16, kind="ExternalInput")
    bq_d = nc.dram_tensor("bq", [D, 1], F32, kind="ExternalInput")
    bk_d = nc.dram_tensor("bk", [D, 1], F32, kind="ExternalInput")
    bvb_d = nc.dram_tensor("bvb", [128, D], F32, kind="ExternalInput")
    lng_d = nc.dram_tensor("lng", [128, D], F32, kind="ExternalInput")
    lnb_d = nc.dram_tensor("lnb", [128, D], F32, kind="ExternalInput")
    ones_d = nc.dram_tensor("ones32", [128, DH], BF16, kind="ExternalInput")
    maskm_d = nc.dram_tensor("maskm", [N, N], BF16, kind="ExternalInput")
    out_d = nc.dram_tensor("out", [PER_CORE, N, D], F32, kind="ExternalOutput")

    with tile.TileContext(nc) as tc, ExitStack() as ctx:
        const = ctx.enter_context(tc.tile_pool(name="const", bufs=1))
        work = ctx.enter_context(tc.tile_pool(name="work", bufs=2))
        expp = ctx.enter_context(tc.tile_pool(name="expp", bufs=40))
        pst = ctx.enter_context(tc.tile_pool(name="pst", bufs=3, space="PSUM"))
        pdp = ctx.enter_context(tc.tile_pool(name="pdp", bufs=1, space="PSUM"))

        # ---- constants ----
        ta_sb = const.tile([128, N], F32)
        nc.gpsimd.dma_start(ta_sb, ta_d[:, :])
        wq_sb = const.tile([D, D], BF16)
        nc.gpsimd.dma_start(wq_sb, wq_d[:, :])
        wk_sb = const.tile([D, D], BF16)
        nc.gpsimd.dma_start(wk_sb, wk_d[:, :])
        wv_sb = const.tile([D, D], BF16)
        nc.gpsimd.dma_start(wv_sb, wv_d[:, :])
        wo_sb = const.tile([D, D], BF16)
        nc.gpsimd.dma_start(wo_sb, wo_d[:, :])
        bq_sb = const.tile([D, 1], F32)
        nc.gpsimd.dma_start(bq_sb, bq_d[:, :])
        bk_sb = const.tile([D, 1], F32)
        nc.gpsimd.dma_start(bk_sb, bk_d[:, :])
        bvb_sb = const.tile([128, D], F32)
        nc.gpsimd.dma_start(bvb_sb, bvb_d[:, :])
        lng_sb = const.tile([128, D], F32)
        nc.gpsimd.dma_start(lng_sb, lng_d[:, :])
        lnb_sb = const.tile([128, D], F32)
        nc.gpsimd.dma_start(lnb_sb, lnb_d[:, :])
        ones_sb = const.tile([128, DH], BF16)
        nc.gpsimd.dma_start(ones_sb, ones_d[:, :])
        eps_sb = const.tile([128, 1], F32)
        nc.vector.memset(eps_sb, EPS)
        mask_sb = []
        for m in range(NTILE):
            mt = const.tile([128, N], BF16, name=f"mask{m}", tag=f"mask{m}")
            nc.gpsimd.dma_start(mt, maskm_d[m * 128 : (m + 1) * 128, :])
            mask_sb.append(mt)

        # per-pair persistent tiles
        qts, kts, vbs, evs, xpbs, ys, mvs, es = {}, {}, {}, {}, {}, {}, {}, {}

        def stage_P(it):
            """load + enhance + Q/K/V projections for pair `it`."""
            xT_sb = work.tile([128, N], F32, name=f"xT{it}", tag="xT", bufs=2)
            nc.sync.dma_start(xT_sb, xT_d[it])
            xpb_sb = work.tile(
                [128, NTILE, D], F32, name=f"xpb{it}", tag="xpb", bufs=2
            )
            nc.sync.dma_start(
                xpb_sb, xpb_d[it].rearrange("(q p) d -> p q d", p=128)
            )
            xpbs[it] = xpb_sb
            enhT = work.tile([128, N], BF16, name=f"enhT{it}", tag="enhT", bufs=2)
            nc.vector.tensor_add(enhT, xT_sb, ta_sb)

            for nm, w_sb, b_sb in (("q", wq_sb, bq_sb), ("k", wk_sb, bk_sb)):
                ps = pst.tile([128, N], F32, name=f"ps{nm}{it}", tag="st")
                for j in range(2):
                    nc.tensor.matmul(
                        ps[:, j * 512 : (j + 1) * 512],
                        w_sb,
                        enhT[:, j * 512 : (j + 1) * 512],
                        start=True,
                        stop=True,
                    )
                dst = work.tile([128, N], BF16, name=f"{nm}t{it}", tag=f"{nm}t", bufs=2)
                nc.vector.tensor_scalar_add(dst, ps, b_sb)
                if nm == "q":
                    qts[it] = dst
                else:
                    kts[it] = dst

            vb = work.tile([128, NTILE, D], BF16, name=f"vb{it}", tag="vb", bufs=2)
            for m in range(NTILE):
                psv = pst.tile([128, D], F32, name=f"psv{it}_{m}", tag="st")
                nc.tensor.matmul(
                    psv, enhT[:, m * 128 : (m + 1) * 128], wv_sb, start=True, stop=True
                )
                nc.vector.tensor_add(vb[:, m, :], psv, bvb_sb)
            vbs[it] = vb
            evs[it] = work.tile([128, N], BF16, name=f"ev{it}", tag="ev", bufs=2)

        def stage_A(it, j):
            """QK^T + exp + mask for nq half `j` of pair `it`."""
            qt, kt = qts[it], kts[it]
            pool_budget = POOLM
            for m in range(NTILE):
                for hp in range(2):
                    st = pst.tile([128, N], F32, name=f"st{it}_{j}_{m}_{hp}", tag="st")
                    for hh in range(2):
                        h = 2 * hp + hh
                        nc.tensor.matmul(
                            st[:, hh * 512 : (hh + 1) * 512],
                            kt[32 * h : 32 * h + 32, m * 128 : (m + 1) * 128],
                            qt[32 * h : 32 * h + 32, j * 512 : (j + 1) * 512],
                            start=True,
                            stop=True,
                            tile_position=(32 * h, 0),
                        )
                    e = expp.tile(
                        [128, N], BF16, name=f"e{it}_{j}_{m}_{hp}", tag="e"
                    )
                    nc.scalar.activation(e, st, AF.Exp)
                    for hh in range(2):
                        eng = nc.vector
                        if pool_budget > 0:
                            eng = nc.gpsimd
                            pool_budget -= 1
                        eng.tensor_mul(
                            e[:, hh * 512 : (hh + 1) * 512],
                            e[:, hh * 512 : (hh + 1) * 512],
                            mask_sb[m][:, j * 512 : (j + 1) * 512],
                        )
                    es[(it, j, m, hp)] = e

        def stage_B(it, j):
            """col-tiled AV + denominator + normalize for (pair, j)."""
            vb, ev = vbs[it], evs[it]
            pd = pdp.tile([128, N], F32, name=f"pd{it}_{j}", tag="pd")
            for m in range(NTILE):
                for h in range(H):
                    e = es[(it, j, m, h // 2)]
                    nc.tensor.matmul(
                        pd[32 * h : 32 * h + 32, 0:512],
                        vb[:, m, 32 * h : 32 * h + 32],
                        e[:, (h % 2) * 512 : (h % 2) * 512 + 512],
                        start=(m == 0),
                        stop=(m == NTILE - 1),
                        tile_position=(0, 32 * h),
                    )
                for h in range(H):
                    e = es[(it, j, m, h // 2)]
                    nc.tensor.matmul(
                        pd[32 * h : 32 * h + 32, 512:1024],
                        ones_sb,
                        e[:, (h % 2) * 512 : (h % 2) * 512 + 512],
                        start=(m == 0),
                        stop=(m == NTILE - 1),
                        tile_position=(0, 32 * h),
                    )
            rec = work.tile([128, 512], F32, name=f"rec{it}_{j}", tag="rec", bufs=2)
            nc.vector.reciprocal(rec, pd[:, 512:1024])
            nc.vector.tensor_mul(ev[:, j * 512 : (j + 1) * 512], pd[:, 0:512], rec)

        def stage_O(it):
            """Wo projection (stationary-swap -> natural) + residual + stats."""
            ev, xpb_sb = evs[it], xpbs[it]
            y = work.tile([128, NTILE, D], F32, name=f"y{it}", tag=f"y{it}", bufs=1)
            mv = work.tile([128, NTILE, 2], F32, name=f"mv{it}", tag=f"mv{it}", bufs=1)
            for c in range(NTILE):
                pso = pst.tile([128, D], F32, name=f"pso{it}_{c}", tag="st")
                nc.tensor.matmul(
                    pso, ev[:, c * 128 : (c + 1) * 128], wo_sb, start=True, stop=True
                )
                nc.vector.tensor_add(y[:, c, :], pso, xpb_sb[:, c, :])
                st6 = work.tile([128, 6], F32, name=f"st6{it}_{c}", tag="st6", bufs=8)
                nc.vector.bn_stats(st6, y[:, c, :])
                nc.vector.bn_aggr(mv[:, c, :], st6)
            ys[it], mvs[it] = y, mv

        def stage_LN(it):
            """deferred LayerNorm tail + store."""
            y, mv = ys[it], mvs[it]
            sd = work.tile([128, NTILE, 1], F32, name=f"sd{it}", tag="sd", bufs=2)
            nc.scalar.activation(sd, mv[:, :, 1:2], AF.Sqrt, bias=eps_sb[:, 0:1])
            rstd = work.tile([128, NTILE, 1], F32, name=f"rstd{it}", tag="rstd", bufs=2)
            nc.vector.reciprocal(rstd, sd)
            oall = work.tile([128, NTILE, D], F32, name=f"oall{it}", tag="oall", bufs=2)
            for c in range(NTILE):
                z = work.tile([128, D], F32, name=f"z{it}_{c}", tag="z", bufs=4)
                nc.vector.tensor_scalar(
                    z,
                    y[:, c, :],
                    mv[:, c, 0:1],
                    rstd[:, c, 0:1],
                    op0=mybir.AluOpType.subtract,
                    op1=mybir.AluOpType.mult,
                )
                nc.vector.tensor_mul(z, z, lng_sb)
                nc.vector.tensor_add(oall[:, c, :], z, lnb_sb)
            nc.sync.dma_start(out_d[it].rearrange("(q p) d -> p q d", p=128), oall)

        # ---- half-pair software pipeline ----
        # P0 A00 A01 B00 P1 A10 B01 O0 A11 B10 P2 A20 B11 O1 A21 B20 B21 O2
        stage_P(0)
        stage_A(0, 0)
        stage_A(0, 1)
        stage_B(0, 0)
        stage_P(1)
        stage_A(1, 0)
        stage_B(0, 1)
        stage_O(0)
        stage_A(1, 1)
        stage_B(1, 0)
        stage_P(2)
        stage_A(2, 0)
        stage_B(1, 1)
        stage_O(1)
        stage_A(2, 1)
        stage_B(2, 0)
        stage_B(2, 1)
        stage_O(2)
        for it in range(PER_CORE):
            stage_LN(it)

    nc.compile()
    return nc


_nc_cache = {}


def _get_nc():
    key = POOLM
    if key not in _nc_cache:
        _nc_cache[key] = _build_nc()
    return _nc_cache[key]


def kernel(
    node_features,
    adj_mx,
    node_type_embed,
    Wq,
    bq,
    Wk,
    bk,
    Wv,
    bv,
    edge_bias,
    Wo,
    bo,
    ln_g,
    ln_b,
):
    global LAST_RESULTS
    nf = np.asarray(node_features, np.float32)
    adj = np.asarray(adj_mx)
    nte = np.asarray(node_type_embed, np.float32)
    Wq = np.asarray(Wq, np.float32)
    Wk = np.asarray(Wk, np.float32)
    Wv = np.asarray(Wv, np.float32)
    Wo = np.asarray(Wo, np.float32)
    bq = np.asarray(bq, np.float32)
    bk = np.asarray(bk, np.float32)
    bv = np.asarray(bv, np.float32)
    bo = np.asarray(bo, np.float32)
    edge_bias = np.asarray(edge_bias, np.float32)
    ln_g = np.asarray(ln_g, np.float32)
    ln_b = np.asarray(ln_b, np.float32)

    scale = 1.0 / np.sqrt(DH)

    # shared (replicated) inputs
    types = 1 - (np.arange(N) % 2)
    ta = np.ascontiguousarray(nte[types].T)  # (D, N)
    keep = np.maximum(adj.astype(np.float32), np.eye(N, dtype=np.float32))
    maskmul = np.ascontiguousarray((np.exp(edge_bias) * keep).T)  # (m, nq)
    shared = {
        "ta": ta,
        "wq": np.ascontiguousarray(Wq.T * scale).astype(BF16_NP),
        "wk": np.ascontiguousarray(Wk.T).astype(BF16_NP),
        "wv": np.ascontiguousarray(Wv.T).astype(BF16_NP),
        "wo": np.ascontiguousarray(Wo.T).astype(BF16_NP),
        "bq": np.ascontiguousarray((bq * scale).reshape(D, 1)),
        "bk": np.ascontiguousarray(bk.reshape(D, 1)),
        "bvb": np.ascontiguousarray(np.broadcast_to(bv, (128, D))),
        "lng": np.ascontiguousarray(np.broadcast_to(ln_g, (128, D))),
        "lnb": np.ascontiguousarray(np.broadcast_to(ln_b, (128, D))),
        "ones32": np.ones((128, DH), BF16_NP),
        "maskm": maskmul.astype(BF16_NP),
    }

    in_maps = []
    for c in range(NCORES):
        pairs = PAIRS[c * PER_CORE : (c + 1) * PER_CORE]
        xT = np.stack([np.ascontiguousarray(nf[b, :, t, :].T) for (b, t) in pairs])
        xpb = np.stack([nf[b, :, t, :] + bo for (b, t) in pairs])
        in_maps.append({**shared, "xt": xT, "xpb": xpb})

    nc = _get_nc()
    res = run_bass_kernel_spmd(
        nc,
        in_maps,
        core_ids=list(range(NCORES)),
        trace=bool(int(os.environ.get("BASSK_TRACE", "0"))),
    )
    LAST_RESULTS = res

    out = np.empty((B, N, T, D), np.float32)
    for c in range(NCORES):
        pairs = PAIRS[c * PER_CORE : (c + 1) * PER_CORE]
        for i, (b, t) in enumerate(pairs):
            out[b, :, t, :] = res.results[c]["out"][i]
    return out


# revision 3
# speedup vs baseline: 1.3058x; 1.1502x over previous
"""Trainium2 Bass kernel for nn_CHGANSimplified (sparse graph attention).

Math (per batch b, time t):
  enh = x + type_embed[parity(n)]
  Q/K/V = enh @ W*.T + b*          (4 heads, head dim 32)
  S_h = (Q_h K_h^T)/sqrt(32) + edge_bias ; masked where adj==0 & ~eye
  out = LN(concat_h(softmax(S_h) V_h) @ Wo.T + bo + x)

Device strategy (8 cores, data-parallel over the 24 (b,t) pairs, 3 each).
Per-core schedule (the ACT engine's exp stream is the critical resource):

  - feature-major layout: enhT/qt/kt (D=128 part, N=1024); head h sits
    at partitions 32h..32h+31.
  - QK^T: row-tiled PE (K=32/head, tile_position=(32h,0)); exp off PSUM
    on ACT -> e bf16; multiplicative mask (0/1 incl diag), one DVE mul
    per [128,1024] chunk via a column-doubled mask layout.
  - AV: col-tiled PE (V_h natural [m,32] stationary, tile (0,32h)),
    feature-major output; denominator via all-ones [128,32] stationary
    in the same pass (per-lane broadcast rowsums); fast-reciprocal+mul.
  - Wo: stationary-swap (evT chunk stationary) -> natural output for
    residual + LayerNorm; LN sqrt deferred to one tail (one ACT table
    switch total).
  - PE runs in-order and mode switches (row<->col tiling) cost ~650ns,
    so AV/proj/Wo blocks are fenced behind a PE nop that reads the last
    QK st tile of the surrounding exp window: each block executes as
    one solid excursion in the window's tail instead of thrashing modes.
"""

import os
import sys

sys.path.insert(0, "/opt/trn_rl_repo")

from contextlib import ExitStack

import ml_dtypes
import numpy as np

import concourse.bass as bass
import concourse.tile as tile
from concourse import bacc, mybir
from concourse.bass_utils import run_bass_kernel_spmd

B, N, T, D, H, DH = 2, 1024, 12, 128, 4, 32
NCORES = 8
PAIRS = [(b, t) for b in range(B) for t in range(T)]
PER_CORE = len(PAIRS) // NCORES  # 3
EPS = 1e-5
NTILE = N // 128  # 8

BF16, BF16_NP = mybir.dt.bfloat16, ml_dtypes.bfloat16
F32 = mybir.dt.float32
AF = mybir.ActivationFunctionType

# number of (m,hp) mask-mul chunks per (pair,j) stage routed to gpsimd
POOLM = int(os.environ.get("BASSK_POOLM", "2"))

LAST_RESULTS = None  # BassKernelResults of the most recent run (for test.py)

CBF_W = 4 * 128 + DH  # wq|wk|wv|wo|ones32
CF_W = 1 + 1 + 3 * 128  # bq|bk|bvb|lng|lnb


def _build_nc():
    nc = bacc.Bacc()

    xT_d = nc.dram_tensor("xt", [PER_CORE, 128, N], F32, kind="ExternalInput")
    xpb_d = nc.dram_tensor("xpb", [PER_CORE, N, D], F32, kind="ExternalInput")
    ta_d = nc.dram_tensor("ta", [128, N], F32, kind="ExternalInput")
    cbf_d = nc.dram_tensor("cbf", [128, CBF_W], BF16, kind="ExternalInput")
    cf_d = nc.dram_tensor("cf", [128, CF_W], F32, kind="ExternalInput")
    maskd_d = nc.dram_tensor("maskd", [N, 2 * N], BF16, kind="ExternalInput")
    out_d = nc.dram_tensor("out", [PER_CORE, N, D], F32, kind="ExternalOutput")

    with tile.TileContext(nc) as tc, ExitStack() as ctx:
        const = ctx.enter_context(tc.tile_pool(name="const", bufs=1))
        work = ctx.enter_context(tc.tile_pool(name="work", bufs=2))
        expp = ctx.enter_context(tc.tile_pool(name="expp", bufs=40))
        pst = ctx.enter_context(tc.tile_pool(name="pst", bufs=3, space="PSUM"))
        pdp = ctx.enter_context(tc.tile_pool(name="pdp", bufs=1, space="PSUM"))

        # ---- constants (two consolidated blobs + ta + masks) ----
        ta_sb = const.tile([128, N], F32)
        nc.gpsimd.dma_start(ta_sb, ta_d[:, :])
        cbf_sb = const.tile([128, CBF_W], BF16)
        nc.gpsimd.dma_start(cbf_sb, cbf_d[:, :])
        cf_sb = const.tile([128, CF_W], F32)
        nc.gpsimd.dma_start(cf_sb, cf_d[:, :])
        wq_sb = cbf_sb[:, 0:128]
        wk_sb = cbf_sb[:, 128:256]
        wv_sb = cbf_sb[:, 256:384]
        wo_sb = cbf_sb[:, 384:512]
        ones_sb = cbf_sb[:, 512 : 512 + DH]
        bq_sb = cf_sb[:, 0:1]
        bk_sb = cf_sb[:, 1:2]
        bvb_sb = cf_sb[:, 2:130]
        lng_sb = cf_sb[:, 130:258]
        lnb_sb = cf_sb[:, 258:386]
        eps_sb = const.tile([128, 1], F32)
        nc.vector.memset(eps_sb, EPS)
        mask_sb = []
        for m in range(NTILE):
            mt = const.tile([128, 2 * N], BF16, name=f"mask{m}", tag=f"mask{m}")
            nc.gpsimd.dma_start(mt, maskd_d[m * 128 : (m + 1) * 128, :])
            mask_sb.append(mt)

        qts, kts, vbs, evs, xpbs, ys, mvs, es = {}, {}, {}, {}, {}, {}, {}, {}
        last_st = [None]

        def pe_fence():
            """PE nop reading the newest st tile: blocks later PE work
            (mode-switching excursions) until the current exp window's QK
            stream has fully issued, so excursions run as one solid block."""
            if last_st[0] is None:
                return
            with tc.tile_critical():
                nop = nc.tensor.nop(hint="dep", nofuse=True).ins
                nop.ins = [nc.tensor.lower_ap(last_st[0][:, 0:1])]

        def stage_P(it):
            """load + enhance + Q/K/V projections for pair `it`."""
            xT_sb = work.tile([128, N], F32, name=f"xT{it}", tag="xT", bufs=2)
            nc.sync.dma_start(xT_sb, xT_d[it])
            xpb_sb = work.tile([128, NTILE, D], F32, name=f"xpb{it}", tag="xpb", bufs=2)
            nc.sync.dma_start(xpb_sb, xpb_d[it].rearrange("(q p) d -> p q d", p=128))
            xpbs[it] = xpb_sb
            enhT = work.tile([128, N], BF16, name=f"enhT{it}", tag="enhT", bufs=2)
            nc.vector.tensor_add(enhT, xT_sb, ta_sb)

            for nm, w_sb, b_sb in (("q", wq_sb, bq_sb), ("k", wk_sb, bk_sb)):
                ps = pst.tile([128, N], F32, name=f"ps{nm}{it}", tag="st")
                for j in range(2):
                    nc.tensor.matmul(
                        ps[:, j * 512 : (j + 1) * 512],
                        w_sb,
                        enhT[:, j * 512 : (j + 1) * 512],
                        start=True,
                        stop=True,
                    )
                dst = work.tile([128, N], BF16, name=f"{nm}t{it}", tag=f"{nm}t", bufs=2)
                nc.vector.tensor_scalar_add(dst, ps, b_sb)
                if nm == "q":
                    qts[it] = dst
                else:
                    kts[it] = dst

            vb = work.tile([128, NTILE, D], BF16, name=f"vb{it}", tag="vb", bufs=2)
            for m in range(NTILE):
                psv = pst.tile([128, D], F32, name=f"psv{it}_{m}", tag="st")
                nc.tensor.matmul(
                    psv, enhT[:, m * 128 : (m + 1) * 128], wv_sb, start=True, stop=True
                )
                nc.vector.tensor_add(vb[:, m, :], psv, bvb_sb)
            vbs[it] = vb
            evs[it] = work.tile([128, N], BF16, name=f"ev{it}", tag="ev", bufs=2)

        def stage_A(it, j, hidden=()):
            """QK^T + exp + mask for nq half `j` of pair `it`; then emit
            the fenced excursion blocks under this exp window's tail."""
            qt, kt = qts[it], kts[it]
            chunk = 0
            for m in range(NTILE):
                for hp in range(2):
                    st = pst.tile([128, N], F32, name=f"st{it}_{j}_{m}_{hp}", tag="st")
                    for hh in range(2):
                        h = 2 * hp + hh
                        nc.tensor.matmul(
                            st[:, hh * 512 : (hh + 1) * 512],
                            kt[32 * h : 32 * h + 32, m * 128 : (m + 1) * 128],
                            qt[32 * h : 32 * h + 32, j * 512 : (j + 1) * 512],
                            start=True,
                            stop=True,
                            tile_position=(32 * h, 0),
                        )
                    last_st[0] = st
                    e = expp.tile([128, N], BF16, name=f"e{it}_{j}_{m}_{hp}", tag="e")
                    nc.scalar.activation(e, st, AF.Exp)
                    eng = nc.gpsimd if chunk < POOLM else nc.vector
                    eng.tensor_mul(e, e, mask_sb[m][:, j * N : (j + 1) * N])
                    es[(it, j, m, hp)] = e
                    chunk += 1
            pe_fence()
            for fn in hidden:
                fn()

        def stage_B(it, j):
            """col-tiled AV + denominator + normalize for (pair, j)."""
            vb, ev = vbs[it], evs[it]
            pd = pdp.tile([128, N], F32, name=f"pd{it}_{j}", tag="pd")
            for m in range(NTILE):
                for h in range(H):
                    e = es[(it, j, m, h // 2)]
                    nc.tensor.matmul(
                        pd[32 * h : 32 * h + 32, 0:512],
                        vb[:, m, 32 * h : 32 * h + 32],
                        e[:, (h % 2) * 512 : (h % 2) * 512 + 512],
                        start=(m == 0),
                        stop=(m == NTILE - 1),
                        tile_position=(0, 32 * h),
                    )
                for h in range(H):
                    e = es[(it, j, m, h // 2)]
                    nc.tensor.matmul(
                        pd[32 * h : 32 * h + 32, 512:1024],
                        ones_sb,
                        e[:, (h % 2) * 512 : (h % 2) * 512 + 512],
                        start=(m == 0),
                        stop=(m == NTILE - 1),
                        tile_position=(0, 32 * h),
                    )
            rec = work.tile([128, 512], F32, name=f"rec{it}_{j}", tag="rec", bufs=2)
            nc.vector.reciprocal_approx_fast(rec, pd[:, 512:1024])
            nc.vector.tensor_mul(ev[:, j * 512 : (j + 1) * 512], pd[:, 0:512], rec)

        def stage_O(it):
            """Wo projection (stationary-swap -> natural) + residual + stats."""
            ev, xpb_sb = evs[it], xpbs[it]
            y = work.tile([128, NTILE, D], F32, name=f"y{it}", tag=f"y{it}", bufs=1)
            mv = work.tile([128, NTILE, 2], F32, name=f"mv{it}", tag=f"mv{it}", bufs=1)
            for c in range(NTILE):
                pso = pst.tile([128, D], F32, name=f"pso{it}_{c}", tag="st")
                nc.tensor.matmul(
                    pso, ev[:, c * 128 : (c + 1) * 128], wo_sb, start=True, stop=True
                )
                nc.vector.tensor_add(y[:, c, :], pso, xpb_sb[:, c, :])
                st6 = work.tile([128, 6], F32, name=f"st6{it}_{c}", tag="st6", bufs=8)
                nc.vector.bn_stats(st6, y[:, c, :])
                nc.vector.bn_aggr(mv[:, c, :], st6)
            ys[it], mvs[it] = y, mv

        def stage_LN(it):
            """deferred LayerNorm tail + store."""
            y, mv = ys[it], mvs[it]
            sd = work.tile([128, NTILE, 1], F32, name=f"sd{it}", tag="sd", bufs=2)
            nc.scalar.activation(sd, mv[:, :, 1:2], AF.Sqrt, bias=eps_sb[:, 0:1])
            rstd = work.tile([128, NTILE, 1], F32, name=f"rstd{it}", tag="rstd", bufs=2)
            nc.vector.reciprocal(rstd, sd)
            oall = work.tile([128, NTILE, D], F32, name=f"oall{it}", tag="oall", bufs=2)
            for c in range(NTILE):
                z = work.tile([128, D], F32, name=f"z{it}_{c}", tag="z", bufs=4)
                nc.vector.tensor_scalar(
                    z,
                    y[:, c, :],
                    mv[:, c, 0:1],
                    rstd[:, c, 0:1],
                    op0=mybir.AluOpType.subtract,
                    op1=mybir.AluOpType.mult,
                )
                nc.vector.tensor_mul(z, z, lng_sb)
                nc.vector.tensor_add(oall[:, c, :], z, lnb_sb)
            nc.sync.dma_start(out_d[it].rearrange("(q p) d -> p q d", p=128), oall)

        # ---- half-pair software pipeline with fenced excursions ----
        stage_P(0)
        stage_P(1)
        stage_A(0, 0)
        stage_A(0, 1, [lambda: stage_B(0, 0)])
        stage_A(1, 0, [lambda: stage_B(0, 1), lambda: stage_O(0)])
        stage_A(1, 1, [lambda: stage_B(1, 0), lambda: stage_P(2)])
        stage_A(2, 0, [lambda: stage_B(1, 1), lambda: stage_O(1)])
        stage_A(2, 1, [lambda: stage_B(2, 0)])
        stage_B(2, 1)
        stage_O(2)
        for it in range(PER_CORE):
            stage_LN(it)

    nc.compile()
    return nc


_nc_cache = {}


def _get_nc():
    key = POOLM
    if key not in _nc_cache:
        _nc_cache[key] = _build_nc()
    return _nc_cache[key]


def kernel(
    node_features,
    adj_mx,
    node_type_embed,
    Wq,
    bq,
    Wk,
    bk,
    Wv,
    bv,
    edge_bias,
    Wo,
    bo,
    ln_g,
    ln_b,
):
    global LAST_RESULTS
    nf = np.asarray(node_features, np.float32)
    adj = np.asarray(adj_mx)
    nte = np.asarray(node_type_embed, np.float32)
    Wq = np.asarray(Wq, np.float32)
    Wk = np.asarray(Wk, np.float32)
    Wv = np.asarray(Wv, np.float32)
    Wo = np.asarray(Wo, np.float32)
    bq = np.asarray(bq, np.float32)
    bk = np.asarray(bk, np.float32)
    bv = np.asarray(bv, np.float32)
    bo = np.asarray(bo, np.float32)
    edge_bias = np.asarray(edge_bias, np.float32)
    ln_g = np.asarray(ln_g, np.float32)
    ln_b = np.asarray(ln_b, np.float32)

    scale = 1.0 / np.sqrt(DH)

    # shared (replicated) inputs
    types = 1 - (np.arange(N) % 2)
    ta = np.ascontiguousarray(nte[types].T)  # (D, N)
    keep = np.maximum(adj.astype(np.float32), np.eye(N, dtype=np.float32))
    mm = (np.exp(edge_bias) * keep).T.astype(BF16_NP)  # (m, nq)
    # column-doubled mask: [j0 | j0 | j1 | j1] so one [128,1024] mul covers
    # both heads of a pair (e layout is [h0 512 | h1 512] per nq half)
    maskd = np.concatenate(
        [mm[:, 0:512], mm[:, 0:512], mm[:, 512:1024], mm[:, 512:1024]], axis=1
    )
    cbf = np.concatenate(
        [
            (Wq.T * scale).astype(BF16_NP),
            Wk.T.astype(BF16_NP),
            Wv.T.astype(BF16_NP),
            Wo.T.astype(BF16_NP),
            np.ones((128, DH), BF16_NP),
        ],
        axis=1,
    )
    cf = np.concatenate(
        [
            (bq * scale).reshape(D, 1),
            bk.reshape(D, 1),
            np.broadcast_to(bv, (128, D)),
            np.broadcast_to(ln_g, (128, D)),
            np.broadcast_to(ln_b, (128, D)),
        ],
        axis=1,
    ).astype(np.float32)
    shared = {
        "ta": ta,
        "cbf": np.ascontiguousarray(cbf),
        "cf": np.ascontiguousarray(cf),
        "maskd": np.ascontiguousarray(maskd),
    }

    in_maps = []
    for c in range(NCORES):
        pairs = PAIRS[c * PER_CORE : (c + 1) * PER_CORE]
        xT = np.stack([np.ascontiguousarray(nf[b, :, t, :].T) for (b, t) in pairs])
        xpb = np.stack([nf[b, :, t, :] + bo for (b, t) in pairs])
        in_maps.append({**shared, "xt": xT, "xpb": xpb})

    nc = _get_nc()
    res = run_bass_kernel_spmd(
        nc,
        in_maps,
        core_ids=list(range(NCORES)),
        trace=bool(int(os.environ.get("BASSK_TRACE", "0"))),
    )
    LAST_RESULTS = res

    out = np.empty((B, N, T, D), np.float32)
    for c in range(NCORES):
        pairs = PAIRS[c * PER_CORE : (c + 1) * PER_CORE]
        for i, (b, t) in enumerate(pairs):
            out[b, :, t, :] = res.results[c]["out"][i]
    return out


# revision 14
# speedup vs baseline: 1.3485x; 1.0327x over previous
"""Trainium2 Bass kernel for nn_CHGANSimplified (sparse graph attention).

Math (per batch b, time t):
  enh = x + type_embed[parity(n)]
  Q/K/V = enh @ W*.T + b*          (4 heads, head dim 32)
  S_h = (Q_h K_h^T)/sqrt(32) + edge_bias ; masked where adj==0 & ~eye
  out = LN(concat_h(softmax(S_h) V_h) @ Wo.T + bo + x)

Device strategy (8 cores, data-parallel over the 24 (b,t) pairs, 3 each).
Per-core schedule (the ACT engine's exp stream is the critical resource):

  - feature-major layout: enhT/qt/kt (D=128 part, N=1024); head h sits
    at partitions 32h..32h+31.
  - QK^T: row-tiled PE (K=32/head, tile_position=(32h,0)); exp off PSUM
    on ACT -> e bf16; multiplicative mask (0/1 incl diag), one DVE mul
    per [128,1024] chunk via a column-doubled mask layout.
  - AV: col-tiled PE (V_h natural [m,32] stationary, tile (0,32h)),
    feature-major output; denominator via all-ones [128,32] stationary
    in the same pass (per-lane broadcast rowsums); fast-reciprocal+mul.
  - Wo: stationary-swap (evT chunk stationary) -> natural output for
    residual + LayerNorm; LN sqrt deferred to one tail (one ACT table
    switch total).
  - PE runs in-order and mode switches (row<->col tiling) cost ~650ns,
    so AV/proj/Wo blocks are fenced behind a PE nop that reads the last
    QK st tile of the surrounding exp window: each block executes as
    one solid excursion in the window's tail instead of thrashing modes.
"""

import os
import sys

sys.path.insert(0, "/opt/trn_rl_repo")

from contextlib import ExitStack

import ml_dtypes
import numpy as np

import concourse.bass as bass
import concourse.tile as tile
from concourse import bacc, mybir
from concourse.bass_utils import run_bass_kernel_spmd

B, N, T, D, H, DH = 2, 1024, 12, 128, 4, 32
NCORES = 8
PAIRS = [(b, t) for b in range(B) for t in range(T)]
PER_CORE = len(PAIRS) // NCORES  # 3
EPS = 1e-5
NTILE = N // 128  # 8

BF16, BF16_NP = mybir.dt.bfloat16, ml_dtypes.bfloat16
F32 = mybir.dt.float32
AF = mybir.ActivationFunctionType

# number of (m,hp) mask-mul chunks per (pair,j) stage routed to gpsimd
POOLM = int(os.environ.get("BASSK_POOLM", "2"))
# chunk index within a stage_A window at which the fenced excursion runs
FENCE_AT = int(os.environ.get("BASSK_FENCE", "9"))

LAST_RESULTS = None  # BassKernelResults of the most recent run (for test.py)

CBF_W = 4 * 128 + DH  # wq|wk|wv|wo|ones32
CF_W = 1 + 1 + 3 * 128  # bq|bk|bvb|lng|lnb


def _build_nc():
    nc = bacc.Bacc()

    xT_d = nc.dram_tensor("xt", [PER_CORE, 128, N], F32, kind="ExternalInput")
    # host pre-permuted to [128, q, d] so the DMA is contiguous per partition
    xpb_d = nc.dram_tensor("xpb", [PER_CORE, 128, NTILE * D], F32, kind="ExternalInput")
    ta_d = nc.dram_tensor("ta", [128, N], F32, kind="ExternalInput")
    cbf_d = nc.dram_tensor("cbf", [128, CBF_W], BF16, kind="ExternalInput")
    cf_d = nc.dram_tensor("cf", [128, CF_W], F32, kind="ExternalInput")
    maskd_d = nc.dram_tensor("maskd", [N, 2 * N], BF16, kind="ExternalInput")
    out_d = nc.dram_tensor("out", [PER_CORE, 128, NTILE * D], F32, kind="ExternalOutput")

    with tile.TileContext(nc) as tc, ExitStack() as ctx:
        const = ctx.enter_context(tc.tile_pool(name="const", bufs=1))
        work = ctx.enter_context(tc.tile_pool(name="work", bufs=2))
        expp = ctx.enter_context(tc.tile_pool(name="expp", bufs=40))
        pst = ctx.enter_context(tc.tile_pool(name="pst", bufs=3, space="PSUM"))
        pdp = ctx.enter_context(tc.tile_pool(name="pdp", bufs=1, space="PSUM"))

        # ---- constants (two consolidated blobs + ta + masks) ----
        ta_sb = const.tile([128, N], F32)
        nc.gpsimd.dma_start(ta_sb, ta_d[:, :])
        cbf_sb = const.tile([128, CBF_W], BF16)
        nc.gpsimd.dma_start(cbf_sb, cbf_d[:, :])
        cf_sb = const.tile([128, CF_W], F32)
        nc.gpsimd.dma_start(cf_sb, cf_d[:, :])
        wq_sb = cbf_sb[:, 0:128]
        wk_sb = cbf_sb[:, 128:256]
        wv_sb = cbf_sb[:, 256:384]
        wo_sb = cbf_sb[:, 384:512]
        ones_sb = cbf_sb[:, 512 : 512 + DH]
        bq_sb = cf_sb[:, 0:1]
        bk_sb = cf_sb[:, 1:2]
        bvb_sb = cf_sb[:, 2:130]
        lng_sb = cf_sb[:, 130:258]
        lnb_sb = cf_sb[:, 258:386]
        eps_sb = const.tile([128, 1], F32)
        nc.vector.memset(eps_sb, EPS)
        mask_sb = []
        for m in range(NTILE):
            mt = const.tile([128, 2 * N], BF16, name=f"mask{m}", tag=f"mask{m}")
            nc.gpsimd.dma_start(mt, maskd_d[m * 128 : (m + 1) * 128, :])
            mask_sb.append(mt)

        qts, kts, vbs, evs, xpbs, ys, mvs, es = {}, {}, {}, {}, {}, {}, {}, {}
        last_st = [None]

        def pe_fence():
            """PE nop reading the newest st tile: blocks later PE work
            (mode-switching excursions) until the current exp window's QK
            stream has fully issued, so excursions run as one solid block."""
            if last_st[0] is None:
                return
            with tc.tile_critical():
                nop = nc.tensor.nop(hint="dep", nofuse=True).ins
                nop.ins = [nc.tensor.lower_ap(last_st[0][:, 0:1])]

        def load_xpb(it):
            xpb_sb = work.tile([128, NTILE, D], F32, name=f"xpb{it}", tag="xpb", bufs=2)
            nc.sync.dma_start(xpb_sb, xpb_d[it].rearrange("p (q d) -> p q d", q=NTILE))
            xpbs[it] = xpb_sb

        def stage_P(it, with_xpb=False):
            """load + enhance + Q/K/V projections for pair `it`."""
            xT_sb = work.tile([128, N], F32, name=f"xT{it}", tag="xT", bufs=2)
            nc.sync.dma_start(xT_sb, xT_d[it])
            if with_xpb:
                load_xpb(it)
            enhT = work.tile([128, N], BF16, name=f"enhT{it}", tag="enhT", bufs=2)
            nc.vector.tensor_add(enhT, xT_sb, ta_sb)

            for nm, w_sb, b_sb in (("q", wq_sb, bq_sb), ("k", wk_sb, bk_sb)):
                ps = pst.tile([128, N], F32, name=f"ps{nm}{it}", tag="st")
                for j in range(2):
                    nc.tensor.matmul(
                        ps[:, j * 512 : (j + 1) * 512],
                        w_sb,
                        enhT[:, j * 512 : (j + 1) * 512],
                        start=True,
                        stop=True,
                    )
                dst = work.tile([128, N], BF16, name=f"{nm}t{it}", tag=f"{nm}t", bufs=2)
                nc.vector.tensor_scalar_add(dst, ps, b_sb)
                if nm == "q":
                    qts[it] = dst
                else:
                    kts[it] = dst

            vb = work.tile([128, NTILE, D], BF16, name=f"vb{it}", tag="vb", bufs=2)
            for m in range(NTILE):
                psv = pst.tile([128, D], F32, name=f"psv{it}_{m}", tag="st")
                nc.tensor.matmul(
                    psv, enhT[:, m * 128 : (m + 1) * 128], wv_sb, start=True, stop=True
                )
                nc.vector.tensor_add(vb[:, m, :], psv, bvb_sb)
            vbs[it] = vb
            evs[it] = work.tile([128, N], BF16, name=f"ev{it}", tag="ev", bufs=2)

        def stage_A(it, j, hidden=()):
            """QK^T + exp + mask for nq half `j` of pair `it`; the fenced
            excursion blocks are emitted mid-window (chunk FENCE_AT) so
            they finish inside this exp window instead of stalling the
            next one."""
            qt, kt = qts[it], kts[it]
            chunk = 0
            for m in range(NTILE):
                for hp in range(2):
                    st = pst.tile([128, N], F32, name=f"st{it}_{j}_{m}_{hp}", tag="st")
                    for hh in range(2):
                        h = 2 * hp + hh
                        nc.tensor.matmul(
                            st[:, hh * 512 : (hh + 1) * 512],
                            kt[32 * h : 32 * h + 32, m * 128 : (m + 1) * 128],
                            qt[32 * h : 32 * h + 32, j * 512 : (j + 1) * 512],
                            start=True,
                            stop=True,
                            tile_position=(32 * h, 0),
                        )
                    last_st[0] = st
                    e = expp.tile([128, N], BF16, name=f"e{it}_{j}_{m}_{hp}", tag="e")
                    nc.scalar.activation(e, st, AF.Exp)
                    eng = nc.gpsimd if chunk < POOLM else nc.vector
                    eng.tensor_mul(e, e, mask_sb[m][:, j * N : (j + 1) * N])
                    es[(it, j, m, hp)] = e
                    chunk += 1
                    if chunk == FENCE_AT and hidden:
                        pe_fence()
                        for fn in hidden:
                            fn()
            if FENCE_AT >= 16 and hidden:
                pe_fence()
                for fn in hidden:
                    fn()

        def stage_B(it, j):
            """col-tiled AV + denominator + normalize for (pair, j)."""
            vb, ev = vbs[it], evs[it]
            pd = pdp.tile([128, N], F32, name=f"pd{it}_{j}", tag="pd")
            for m in range(NTILE):
                for h in range(H):
                    e = es[(it, j, m, h // 2)]
                    nc.tensor.matmul(
                        pd[32 * h : 32 * h + 32, 0:512],
                        vb[:, m, 32 * h : 32 * h + 32],
                        e[:, (h % 2) * 512 : (h % 2) * 512 + 512],
                        start=(m == 0),
                        stop=(m == NTILE - 1),
                        tile_position=(0, 32 * h),
                    )
                for h in range(H):
                    e = es[(it, j, m, h // 2)]
                    nc.tensor.matmul(
                        pd[32 * h : 32 * h + 32, 512:1024],
                        ones_sb,
                        e[:, (h % 2) * 512 : (h % 2) * 512 + 512],
                        start=(m == 0),
                        stop=(m == NTILE - 1),
                        tile_position=(0, 32 * h),
                    )
            rec = work.tile([128, 512], F32, name=f"rec{it}_{j}", tag="rec", bufs=2)
            nc.vector.reciprocal_approx_fast(rec, pd[:, 512:1024])
            nc.vector.tensor_mul(ev[:, j * 512 : (j + 1) * 512], pd[:, 0:512], rec)

        def stage_O(it):
            """Wo projection (stationary-swap -> natural) + residual + stats."""
            ev, xpb_sb = evs[it], xpbs[it]
            y = work.tile([128, NTILE, D], F32, name=f"y{it}", tag=f"y{it}", bufs=1)
            mv = work.tile([128, NTILE, 2], F32, name=f"mv{it}", tag=f"mv{it}", bufs=1)
            for c in range(NTILE):
                pso = pst.tile([128, D], F32, name=f"pso{it}_{c}", tag="st")
                nc.tensor.matmul(
                    pso, ev[:, c * 128 : (c + 1) * 128], wo_sb, start=True, stop=True
                )
                nc.vector.tensor_add(y[:, c, :], pso, xpb_sb[:, c, :])
                st6 = work.tile([128, 6], F32, name=f"st6{it}_{c}", tag="st6", bufs=8)
                nc.vector.bn_stats(st6, y[:, c, :])
                nc.vector.bn_aggr(mv[:, c, :], st6)
            ys[it], mvs[it] = y, mv

        def stage_LN(it):
            """LayerNorm + store; rstd = exp(-0.5*ln(var+eps)) keeps the
            ACT table on the natural_log_exp set (no sqrt table switch)."""
            y, mv = ys[it], mvs[it]
            sd = work.tile([128, NTILE, 1], F32, name=f"sd{it}", tag="sd", bufs=2)
            nc.scalar.activation(sd, mv[:, :, 1:2], AF.Ln, bias=eps_sb[:, 0:1])
            rstd = work.tile([128, NTILE, 1], F32, name=f"rstd{it}", tag="rstd", bufs=2)
            nc.scalar.activation(rstd, sd, AF.Exp, scale=-0.5)
            oall = work.tile([128, NTILE, D], F32, name=f"oall{it}", tag="oall", bufs=2)
            for c in range(NTILE):
                z = work.tile([128, D], F32, name=f"z{it}_{c}", tag="z", bufs=4)
                nc.vector.tensor_scalar(
                    z,
                    y[:, c, :],
                    mv[:, c, 0:1],
                    rstd[:, c, 0:1],
                    op0=mybir.AluOpType.subtract,
                    op1=mybir.AluOpType.mult,
                )
                nc.vector.tensor_mul(z, z, lng_sb)
                nc.vector.tensor_add(oall[:, c, :], z, lnb_sb)
            nc.sync.dma_start(out_d[it].rearrange("p (q d) -> p q d", q=NTILE), oall)

        # ---- half-pair software pipeline with fenced excursions ----
        stage_P(0)
        stage_P(1)
        stage_A(0, 0, [lambda: load_xpb(0)])
        stage_A(0, 1, [lambda: stage_B(0, 0), lambda: load_xpb(1)])
        stage_A(1, 0, [lambda: stage_B(0, 1), lambda: stage_O(0)])
        stage_A(1, 1, [lambda: stage_B(1, 0), lambda: stage_P(2, True), lambda: stage_LN(0)])
        stage_A(2, 0, [lambda: stage_B(1, 1), lambda: stage_O(1)])
        stage_A(2, 1, [lambda: stage_B(2, 0), lambda: stage_LN(1)])
        stage_B(2, 1)
        stage_O(2)
        stage_LN(2)

    nc.compile()
    return nc


_nc_cache = {}


def _get_nc():
    key = (POOLM, FENCE_AT)
    if key not in _nc_cache:
        _nc_cache[key] = _build_nc()
    return _nc_cache[key]


def kernel(
    node_features,
    adj_mx,
    node_type_embed,
    Wq,
    bq,
    Wk,
    bk,
    Wv,
    bv,
    edge_bias,
    Wo,
    bo,
    ln_g,
    ln_b,
):
    global LAST_RESULTS
    nf = np.asarray(node_features, np.float32)
    adj = np.asarray(adj_mx)
    nte = np.asarray(node_type_embed, np.float32)
    Wq = np.asarray(Wq, np.float32)
    Wk = np.asarray(Wk, np.float32)
    Wv = np.asarray(Wv, np.float32)
    Wo = np.asarray(Wo, np.float32)
    bq = np.asarray(bq, np.float32)
    bk = np.asarray(bk, np.float32)
    bv = np.asarray(bv, np.float32)
    bo = np.asarray(bo, np.float32)
    edge_bias = np.asarray(edge_bias, np.float32)
    ln_g = np.asarray(ln_g, np.float32)
    ln_b = np.asarray(ln_b, np.float32)

    scale = 1.0 / np.sqrt(DH)

    # shared (replicated) inputs
    types = 1 - (np.arange(N) % 2)
    ta = np.ascontiguousarray(nte[types].T)  # (D, N)
    keep = np.maximum(adj.astype(np.float32), np.eye(N, dtype=np.float32))
    mm = (np.exp(edge_bias) * keep).T.astype(BF16_NP)  # (m, nq)
    # column-doubled mask: [j0 | j0 | j1 | j1] so one [128,1024] mul covers
    # both heads of a pair (e layout is [h0 512 | h1 512] per nq half)
    maskd = np.concatenate(
        [mm[:, 0:512], mm[:, 0:512], mm[:, 512:1024], mm[:, 512:1024]], axis=1
    )
    cbf = np.concatenate(
        [
            (Wq.T * scale).astype(BF16_NP),
            Wk.T.astype(BF16_NP),
            Wv.T.astype(BF16_NP),
            Wo.T.astype(BF16_NP),
            np.ones((128, DH), BF16_NP),
        ],
        axis=1,
    )
    cf = np.concatenate(
        [
            (bq * scale).reshape(D, 1),
            bk.reshape(D, 1),
            np.broadcast_to(bv, (128, D)),
            np.broadcast_to(ln_g, (128, D)),
            np.broadcast_to(ln_b, (128, D)),
        ],
        axis=1,
    ).astype(np.float32)
    shared = {
        "ta": ta,
        "cbf": np.ascontiguousarray(cbf),
        "cf": np.ascontiguousarray(cf),
        "maskd": np.ascontiguousarray(maskd),
    }

    in_maps = []
    for c in range(NCORES):
        pairs = PAIRS[c * PER_CORE : (c + 1) * PER_CORE]
        xT = np.stack([np.ascontiguousarray(nf[b, :, t, :].T) for (b, t) in pairs])
        # [q*128+p, d] -> [p, q*d] so the device DMA is contiguous/partition
        xpb = np.stack(
            [
                (nf[b, :, t, :] + bo)
                .reshape(NTILE, 128, D)
                .transpose(1, 0, 2)
                .reshape(128, NTILE * D)
                for (b, t) in pairs
            ]
        )
        in_maps.append({**shared, "xt": xT, "xpb": np.ascontiguousarray(xpb)})

    nc = _get_nc()
    res = run_bass_kernel_spmd(
        nc,
        in_maps,
        core_ids=list(range(NCORES)),
        trace=bool(int(os.environ.get("BASSK_TRACE", "0"))),
    )
    LAST_RESULTS = res

    out = np.empty((B, N, T, D), np.float32)
    for c in range(NCORES):
        pairs = PAIRS[c * PER_CORE : (c + 1) * PER_CORE]
        for i, (b, t) in enumerate(pairs):
            o = res.results[c]["out"][i].reshape(128, NTILE, D)
            out[b, :, t, :] = o.transpose(1, 0, 2).reshape(N, D)
    return out


# revision 17
# speedup vs baseline: 1.3658x; 1.0128x over previous
"""Trainium2 Bass kernel for nn_CHGANSimplified (sparse graph attention).

Math (per batch b, time t):
  enh = x + type_embed[parity(n)]
  Q/K/V = enh @ W*.T + b*          (4 heads, head dim 32)
  S_h = (Q_h K_h^T)/sqrt(32) + edge_bias ; masked where adj==0 & ~eye
  out = LN(concat_h(softmax(S_h) V_h) @ Wo.T + bo + x)

Device strategy (8 cores, data-parallel over the 24 (b,t) pairs, 3 each).
Per-core schedule (the ACT engine's exp stream is the critical resource):

  - feature-major layout: enhT/qt/kt (D=128 part, N=1024); head h sits
    at partitions 32h..32h+31.
  - QK^T: row-tiled PE (K=32/head, tile_position=(32h,0)); exp off PSUM
    on ACT -> e bf16; multiplicative mask (0/1 incl diag), one DVE mul
    per [128,1024] chunk via a column-doubled mask layout.
  - AV: col-tiled PE (V_h natural [m,32] stationary, tile (0,32h)),
    feature-major output; denominator via all-ones [128,32] stationary
    in the same pass (per-lane broadcast rowsums); fast-reciprocal+mul.
  - Wo: stationary-swap (evT chunk stationary) -> natural output for
    residual + LayerNorm; LN sqrt deferred to one tail (one ACT table
    switch total).
  - PE runs in-order and mode switches (row<->col tiling) cost ~650ns,
    so AV/proj/Wo blocks are fenced behind a PE nop that reads the last
    QK st tile of the surrounding exp window: each block executes as
    one solid excursion in the window's tail instead of thrashing modes.
"""

import os
import sys

sys.path.insert(0, "/opt/trn_rl_repo")

from contextlib import ExitStack

import ml_dtypes
import numpy as np

import concourse.bass as bass
import concourse.tile as tile
from concourse import bacc, mybir
from concourse.bass_utils import run_bass_kernel_spmd

B, N, T, D, H, DH = 2, 1024, 12, 128, 4, 32
NCORES = 8
PAIRS = [(b, t) for b in range(B) for t in range(T)]
PER_CORE = len(PAIRS) // NCORES  # 3
EPS = 1e-5
NTILE = N // 128  # 8

BF16, BF16_NP = mybir.dt.bfloat16, ml_dtypes.bfloat16
F32 = mybir.dt.float32
AF = mybir.ActivationFunctionType

# number of (m,hp) mask-mul chunks per (pair,j) stage routed to gpsimd
POOLM = int(os.environ.get("BASSK_POOLM", "2"))
# chunk index within a stage_A window at which the fenced excursion runs
FENCE_AT = int(os.environ.get("BASSK_FENCE", "9"))

LAST_RESULTS = None  # BassKernelResults of the most recent run (for test.py)

CBF_W = 4 * 128 + DH  # wq|wk|wv|wo|ones32
CF_W = 1 + 1 + 3 * 128  # bq|bk|bvb|lng|lnb


def _build_nc():
    nc = bacc.Bacc()

    xT_d = nc.dram_tensor("xt", [PER_CORE, 128, N], F32, kind="ExternalInput")
    # host pre-permuted to [128, q, d] so the DMA is contiguous per partition
    xpb_d = nc.dram_tensor("xpb", [PER_CORE, 128, NTILE * D], F32, kind="ExternalInput")
    ta_d = nc.dram_tensor("ta", [128, N], F32, kind="ExternalInput")
    cbf_d = nc.dram_tensor("cbf", [128, CBF_W], BF16, kind="ExternalInput")
    cf_d = nc.dram_tensor("cf", [128, CF_W], F32, kind="ExternalInput")
    maskd_d = nc.dram_tensor("maskd", [N, 2 * N], BF16, kind="ExternalInput")
    out_d = nc.dram_tensor("out", [PER_CORE, 128, NTILE * D], F32, kind="ExternalOutput")

    with tile.TileContext(nc) as tc, ExitStack() as ctx:
        const = ctx.enter_context(tc.tile_pool(name="const", bufs=1))
        work = ctx.enter_context(tc.tile_pool(name="work", bufs=2))
        expp = ctx.enter_context(tc.tile_pool(name="expp", bufs=40))
        pst = ctx.enter_context(tc.tile_pool(name="pst", bufs=3, space="PSUM"))
        pdp = ctx.enter_context(tc.tile_pool(name="pdp", bufs=1, space="PSUM"))

        # ---- constants (two consolidated blobs + ta + masks) ----
        ta_sb = const.tile([128, N], F32)
        nc.gpsimd.dma_start(ta_sb, ta_d[:, :])
        cbf_sb = const.tile([128, CBF_W], BF16)
        nc.gpsimd.dma_start(cbf_sb, cbf_d[:, :])
        cf_sb = const.tile([128, CF_W], F32)
        nc.gpsimd.dma_start(cf_sb, cf_d[:, :])
        wq_sb = cbf_sb[:, 0:128]
        wk_sb = cbf_sb[:, 128:256]
        wv_sb = cbf_sb[:, 256:384]
        wo_sb = cbf_sb[:, 384:512]
        ones_sb = cbf_sb[:, 512 : 512 + DH]
        bq_sb = cf_sb[:, 0:1]
        bk_sb = cf_sb[:, 1:2]
        bvb_sb = cf_sb[:, 2:130]
        lng_sb = cf_sb[:, 130:258]
        lnb_sb = cf_sb[:, 258:386]
        eps_sb = const.tile([128, 1], F32)
        nc.vector.memset(eps_sb, EPS)
        mask_sb = []
        for m in range(NTILE):
            mt = const.tile([128, 2 * N], BF16, name=f"mask{m}", tag=f"mask{m}")
            nc.gpsimd.dma_start(mt, maskd_d[m * 128 : (m + 1) * 128, :])
            mask_sb.append(mt)

        qts, kts, vbs, evs, xpbs, ys, mvs, es = {}, {}, {}, {}, {}, {}, {}, {}
        last_st = [None]

        def pe_fence():
            """PE nop reading the newest st tile: blocks later PE work
            (mode-switching excursions) until the current exp window's QK
            stream has fully issued, so excursions run as one solid block."""
            if last_st[0] is None:
                return
            with tc.tile_critical():
                nop = nc.tensor.nop(hint="dep", nofuse=True).ins
                nop.ins = [nc.tensor.lower_ap(last_st[0][:, 0:1])]

        def load_xpb(it):
            xpb_sb = work.tile([128, NTILE, D], F32, name=f"xpb{it}", tag="xpb", bufs=2)
            nc.sync.dma_start(xpb_sb, xpb_d[it].rearrange("p (q d) -> p q d", q=NTILE))
            xpbs[it] = xpb_sb

        def stage_P(it, with_xpb=False):
            """load + enhance + Q/K/V projections for pair `it`."""
            xT_sb = work.tile([128, N], F32, name=f"xT{it}", tag="xT", bufs=2)
            nc.sync.dma_start(xT_sb, xT_d[it])
            if with_xpb:
                load_xpb(it)
            enhT = work.tile([128, N], BF16, name=f"enhT{it}", tag="enhT", bufs=2)
            nc.vector.tensor_add(enhT, xT_sb, ta_sb)

            for nm, w_sb, b_sb in (("q", wq_sb, bq_sb), ("k", wk_sb, bk_sb)):
                ps = pst.tile([128, N], F32, name=f"ps{nm}{it}", tag="st")
                for j in range(2):
                    nc.tensor.matmul(
                        ps[:, j * 512 : (j + 1) * 512],
                        w_sb,
                        enhT[:, j * 512 : (j + 1) * 512],
                        start=True,
                        stop=True,
                    )
                dst = work.tile([128, N], BF16, name=f"{nm}t{it}", tag=f"{nm}t", bufs=2)
                nc.vector.tensor_scalar_add(dst, ps, b_sb)
                if nm == "q":
                    qts[it] = dst
                else:
                    kts[it] = dst

            vb = work.tile([128, NTILE, D], BF16, name=f"vb{it}", tag="vb", bufs=2)
            for m in range(NTILE):
                psv = pst.tile([128, D], F32, name=f"psv{it}_{m}", tag="st")
                nc.tensor.matmul(
                    psv, enhT[:, m * 128 : (m + 1) * 128], wv_sb, start=True, stop=True
                )
                nc.vector.tensor_add(vb[:, m, :], psv, bvb_sb)
            vbs[it] = vb
            evs[it] = work.tile([128, N], BF16, name=f"ev{it}", tag="ev", bufs=2)

        def stage_A(it, j, hidden=()):
            """QK^T + exp + mask for nq half `j` of pair `it`; the fenced
            excursion blocks are emitted mid-window (chunk FENCE_AT) so
            they finish inside this exp window instead of stalling the
            next one."""
            qt, kt = qts[it], kts[it]
            chunk = 0
            for m in range(NTILE):
                for hp in range(2):
                    st = pst.tile([128, N], F32, name=f"st{it}_{j}_{m}_{hp}", tag="st")
                    for hh in range(2):
                        h = 2 * hp + hh
                        nc.tensor.matmul(
                            st[:, hh * 512 : (hh + 1) * 512],
                            kt[32 * h : 32 * h + 32, m * 128 : (m + 1) * 128],
                            qt[32 * h : 32 * h + 32, j * 512 : (j + 1) * 512],
                            start=True,
                            stop=True,
                            tile_position=(32 * h, 0),
                        )
                    last_st[0] = st
                    e = expp.tile([128, N], BF16, name=f"e{it}_{j}_{m}_{hp}", tag="e")
                    nc.scalar.activation(e, st, AF.Exp)
                    eng = nc.gpsimd if chunk < POOLM else nc.vector
                    eng.tensor_mul(e, e, mask_sb[m][:, j * N : (j + 1) * N])
                    es[(it, j, m, hp)] = e
                    chunk += 1
                    if chunk == FENCE_AT and hidden:
                        pe_fence()
                        for fn in hidden:
                            fn()
            if FENCE_AT >= 16 and hidden:
                pe_fence()
                for fn in hidden:
                    fn()

        def stage_B(it, j):
            """col-tiled AV + denominator + normalize for (pair, j)."""
            vb, ev = vbs[it], evs[it]
            pd = pdp.tile([128, N], F32, name=f"pd{it}_{j}", tag="pd")
            # all AV matmuls first, then all denominator matmuls: the ones
            # stationary then loads once per col tile instead of ping-ponging
            # with V chunks every matmul
            for m in range(NTILE):
                for h in range(H):
                    e = es[(it, j, m, h // 2)]
                    nc.tensor.matmul(
                        pd[32 * h : 32 * h + 32, 0:512],
                        vb[:, m, 32 * h : 32 * h + 32],
                        e[:, (h % 2) * 512 : (h % 2) * 512 + 512],
                        start=(m == 0),
                        stop=(m == NTILE - 1),
                        tile_position=(0, 32 * h),
                    )
            for m in range(NTILE):
                for h in range(H):
                    e = es[(it, j, m, h // 2)]
                    nc.tensor.matmul(
                        pd[32 * h : 32 * h + 32, 512:1024],
                        ones_sb,
                        e[:, (h % 2) * 512 : (h % 2) * 512 + 512],
                        start=(m == 0),
                        stop=(m == NTILE - 1),
                        tile_position=(0, 32 * h),
                    )
            rec = work.tile([128, 512], F32, name=f"rec{it}_{j}", tag="rec", bufs=2)
            nc.vector.reciprocal_approx_fast(rec, pd[:, 512:1024])
            nc.vector.tensor_mul(ev[:, j * 512 : (j + 1) * 512], pd[:, 0:512], rec)

        def stage_O(it):
            """Wo projection (stationary-swap -> natural) + residual + stats."""
            ev, xpb_sb = evs[it], xpbs[it]
            y = work.tile([128, NTILE, D], F32, name=f"y{it}", tag=f"y{it}", bufs=1)
            mv = work.tile([128, NTILE, 2], F32, name=f"mv{it}", tag=f"mv{it}", bufs=1)
            for c in range(NTILE):
                pso = pst.tile([128, D], F32, name=f"pso{it}_{c}", tag="st")
                nc.tensor.matmul(
                    pso, ev[:, c * 128 : (c + 1) * 128], wo_sb, start=True, stop=True
                )
                nc.vector.tensor_add(y[:, c, :], pso, xpb_sb[:, c, :])
                st6 = work.tile([128, 6], F32, name=f"st6{it}_{c}", tag="st6", bufs=8)
                nc.vector.bn_stats(st6, y[:, c, :])
                nc.vector.bn_aggr(mv[:, c, :], st6)
            ys[it], mvs[it] = y, mv

        def stage_LN(it):
            """LayerNorm + store; rstd = exp(-0.5*ln(var+eps)) keeps the
            ACT table on the natural_log_exp set (no sqrt table switch)."""
            y, mv = ys[it], mvs[it]
            sd = work.tile([128, NTILE, 1], F32, name=f"sd{it}", tag="sd", bufs=2)
            nc.scalar.activation(sd, mv[:, :, 1:2], AF.Sqrt, bias=eps_sb[:, 0:1])
            rstd = work.tile([128, NTILE, 1], F32, name=f"rstd{it}", tag="rstd", bufs=2)
            nc.vector.reciprocal(rstd, sd)
            oall = work.tile([128, NTILE, D], F32, name=f"oall{it}", tag="oall", bufs=2)
            for c in range(NTILE):
                z = work.tile([128, D], F32, name=f"z{it}_{c}", tag="z", bufs=4)
                nc.vector.tensor_scalar(
                    z,
                    y[:, c, :],
                    mv[:, c, 0:1],
                    rstd[:, c, 0:1],
                    op0=mybir.AluOpType.subtract,
                    op1=mybir.AluOpType.mult,
                )
                nc.vector.tensor_mul(z, z, lng_sb)
                nc.vector.tensor_add(oall[:, c, :], z, lnb_sb)
            nc.sync.dma_start(out_d[it].rearrange("p (q d) -> p q d", q=NTILE), oall)

        # ---- half-pair software pipeline with fenced excursions ----
        stage_P(0)
        stage_A(0, 0, [lambda: stage_P(1), lambda: load_xpb(0)])
        stage_A(0, 1, [lambda: stage_B(0, 0), lambda: load_xpb(1)])
        stage_A(1, 0, [lambda: stage_B(0, 1), lambda: stage_O(0)])
        stage_A(1, 1, [lambda: stage_B(1, 0), lambda: stage_P(2, True)])
        stage_A(2, 0, [lambda: stage_B(1, 1), lambda: stage_O(1)])
        stage_A(2, 1, [lambda: stage_B(2, 0)])
        stage_B(2, 1)
        stage_O(2)
        for it in range(PER_CORE):
            stage_LN(it)

    nc.compile()
    return nc


_nc_cache = {}


def _get_nc():
    key = (POOLM, FENCE_AT)
    if key not in _nc_cache:
        _nc_cache[key] = _build_nc()
    return _nc_cache[key]


def kernel(
    node_features,
    adj_mx,
    node_type_embed,
    Wq,
    bq,
    Wk,
    bk,
    Wv,
    bv,
    edge_bias,
    Wo,
    bo,
    ln_g,
    ln_b,
):
    global LAST_RESULTS
    nf = np.asarray(node_features, np.float32)
    adj = np.asarray(adj_mx)
    nte = np.asarray(node_type_embed, np.float32)
    Wq = np.asarray(Wq, np.float32)
    Wk = np.asarray(Wk, np.float32)
    Wv = np.asarray(Wv, np.float32)
    Wo = np.asarray(Wo, np.float32)
    bq = np.asarray(bq, np.float32)
    bk = np.asarray(bk, np.float32)
    bv = np.asarray(bv, np.float32)
    bo = np.asarray(bo, np.float32)
    edge_bias = np.asarray(edge_bias, np.float32)
    ln_g = np.asarray(ln_g, np.float32)
    ln_b = np.asarray(ln_b, np.float32)

    scale = 1.0 / np.sqrt(DH)

    # shared (replicated) inputs
    types = 1 - (np.arange(N) % 2)
    ta = np.ascontiguousarray(nte[types].T)  # (D, N)
    keep = np.maximum(adj.astype(np.float32), np.eye(N, dtype=np.float32))
    mm = (np.exp(edge_bias) * keep).T.astype(BF16_NP)  # (m, nq)
    # column-doubled mask: [j0 | j0 | j1 | j1] so one [128,1024] mul covers
    # both heads of a pair (e layout is [h0 512 | h1 512] per nq half)
    maskd = np.concatenate(
        [mm[:, 0:512], mm[:, 0:512], mm[:, 512:1024], mm[:, 512:1024]], axis=1
    )
    cbf = np.concatenate(
        [
            (Wq.T * scale).astype(BF16_NP),
            Wk.T.astype(BF16_NP),
            Wv.T.astype(BF16_NP),
            Wo.T.astype(BF16_NP),
            np.ones((128, DH), BF16_NP),
        ],
        axis=1,
    )
    cf = np.concatenate(
        [
            (bq * scale).reshape(D, 1),
            bk.reshape(D, 1),
            np.broadcast_to(bv, (128, D)),
            np.broadcast_to(ln_g, (128, D)),
            np.broadcast_to(ln_b, (128, D)),
        ],
        axis=1,
    ).astype(np.float32)
    shared = {
        "ta": ta,
        "cbf": np.ascontiguousarray(cbf),
        "cf": np.ascontiguousarray(cf),
        "maskd": np.ascontiguousarray(maskd),
    }

    in_maps = []
    for c in range(NCORES):
        pairs = PAIRS[c * PER_CORE : (c + 1) * PER_CORE]
        xT = np.stack([np.ascontiguousarray(nf[b, :, t, :].T) for (b, t) in pairs])
        # [q*128+p, d] -> [p, q*d] so the device DMA is contiguous/partition
        xpb = np.stack(
            [
                (nf[b, :, t, :] + bo)
                .reshape(NTILE, 128, D)
                .transpose(1, 0, 2)
                .reshape(128, NTILE * D)
                for (b, t) in pairs
            ]
        )
        in_maps.append({**shared, "xt": xT, "xpb": np.ascontiguousarray(xpb)})

    nc = _get_nc()
    res = run_bass_kernel_spmd(
        nc,
        in_maps,
        core_ids=list(range(NCORES)),
        trace=bool(int(os.environ.get("BASSK_TRACE", "0"))),
    )
    LAST_RESULTS = res

    out = np.empty((B, N, T, D), np.float32)
    for c in range(NCORES):
        pairs = PAIRS[c * PER_CORE : (c + 1) * PER_CORE]
        for i, (b, t) in enumerate(pairs):
            o = res.results[c]["out"][i].reshape(128, NTILE, D)
            out[b, :, t, :] = o.transpose(1, 0, 2).reshape(N, D)
    return out
